# revision 56
# baseline (speedup 1.0000x reference)
"""MoE top-2 routing kernel for 8 Trainium2 NeuronCores.

Reference computation (per token t):
    gates = softmax(x @ gate_w.T + gate_b)          # [T, E]
    top2 = top_k(gates, 2)
    y[t] = sum_{e in top2} gates[t,e] * (expert_w[e] @ x[t] + expert_b[e])

Sharding: data-parallel over tokens (B*S = 8192 tokens -> 1024 per core),
all expert weights streamed on every core. Everything (gating, softmax,
top-2 selection, expert matmuls, weighted combine) runs on device.

Precision split: gating runs in exact fp32 (top-2 selection flips are the
dominant error source -- a flipped expert costs ~0.3 rel err for that
token), expert matmuls run in bf16 (uniform ~2e-3 noise, well under the
2e-2 budget, 4x the fp32 PE throughput and half the LDWEIGHTS/DMA cost).

Host-side prep only re-lays-out inputs (transposes + dtype casts).
"""

import sys

import numpy as np

try:
    import concourse.bass as bass  # noqa: F401
except ImportError:
    sys.path.insert(0, "/opt/trn_rl_repo")

import ml_dtypes
import concourse.bass as bass
import concourse.mybir as mybir
from concourse.bass_utils import run_bass_kernel_spmd
from concourse.masks import make_identity
from concourse.tile import TileContext

F32 = mybir.dt.float32
BF16 = mybir.dt.bfloat16

P = 128          # partitions
T = 1024         # tokens per core
H = 1024         # hidden
E = 8            # experts
O = 1024         # expert output dim
NT = T // P      # token tiles
NK = H // P      # contraction chunks
NO = O // 512    # output column tiles (512 fp32 = one PSUM bank)

N_CORES = 8

_CACHE = {}


def build_nc():
    nc = bass.Bass(use_seq_codegen=True)
    # This container's walrus build rejects the EVENT_SEMAPHORE_RANGE_CLEAR
    # ISA instruction ("ISA wrong length") that TileContext emits in its exit
    # cleanup. Re-executing a loaded NEFF with dirty semaphores crashes the
    # core, so instead of the range-clear we emit one NoOp per semaphore with
    # a sem-wr-imm(0) update (an instruction form this walrus accepts).
    def _manual_clear(sems, _nc=nc):
        from concourse.bass import compact_to_ranges as _ctr
        nums = [s.num if hasattr(s, "num") else s for s in sems]
        if not nums:
            return
        try:
            for r in _ctr(nums):
                _nc.gpsimd.dma_reset(r)
        except Exception:
            pass
        for n in nums:
            ins = _nc.gpsimd.nop()
            ins.ins.sync_info = mybir.SyncInfo(
                on_wait=[],
                on_update=[mybir.SyncUpdate(
                    sync_type="semaphore", id=n,
                    update_mode="sem-wr-imm", update_value=0)],
            )
    nc.clear_and_free_semaphores = _manual_clear

    xTf = nc.dram_tensor("xTf", [H, T], F32, kind="ExternalInput")
    gwT = nc.dram_tensor("gwT", [H, E], F32, kind="ExternalInput")
    gbc = nc.dram_tensor("gbc", [1, E], F32, kind="ExternalInput")
    wTb = nc.dram_tensor("wTb", [E, P, NK * O], BF16, kind="ExternalInput")
    ebb = nc.dram_tensor("ebb", [E, O], BF16, kind="ExternalInput")
    y = nc.dram_tensor("y", [T, O], F32, kind="ExternalOutput")

    with TileContext(nc) as tc:
        with (
            tc.tile_pool(name="big", bufs=1) as big,
            tc.tile_pool(name="wpool", bufs=2) as wpool,
            tc.tile_pool(name="small", bufs=1) as small,
            tc.tile_pool(name="tmp", bufs=4) as tmpp,
            tc.tile_pool(name="psum", bufs=3, space="PSUM") as psump,
            tc.tile_pool(name="psg", bufs=1, space="PSUM") as psg,
        ):
            # ---- resident tensors ----
            # DMA issue order matters: one serial queue. Small gating tensors
            # first, then the fp32 x chunks (gating consumes them k-major as
            # they land), then bf16 x + expert-0 weights (needed ~20us in).
            gw = small.tile([P, NK * E], F32, tag="gw")
            nc.sync.dma_start(out=gw[:, :], in_=gwT.rearrange("(k p) e -> p k e", p=P))

            gbrow = small.tile([1, E], F32, tag="gbrow")
            nc.sync.dma_start(out=gbrow[:, :], in_=gbc[:, :])
            ones512 = small.tile([1, 512], F32, tag="ones512")
            nc.vector.memset(ones512[:, :], 1.0)

            ebt = small.tile([E, O], BF16, tag="ebt")
            nc.sync.dma_start(out=ebt[:, :], in_=ebb[:, :])

            # fp32 x chunks (gating stationary), chunked so gating starts early
            xg = big.tile([P, NK * T], F32, tag="xg")
            for k in range(NK):
                nc.sync.dma_start(out=xg[:, k * T:(k + 1) * T],
                                  in_=xTf[k * P:(k + 1) * P, :])
            # bf16 x chunks (expert stationary), cast from xg on the Scalar
            # engine (idle during the preamble; keeps the DVE free for the
            # softmax chain and saves 2MB of preamble DMA)
            xbs = big.tile([P, NK * T], BF16, tag="xbs")
            for k in range(NK):
                nc.scalar.copy(xbs[:, k * T:(k + 1) * T],
                               xg[:, k * T:(k + 1) * T])

            ident = small.tile([P, P], F32, tag="ident")
            make_identity(nc, ident[:, :])

            wgt = small.tile([P, NT * E], F32, tag="wgt")    # top-2 gate weights [t, e] per tile
            wgtTb = small.tile([E, T], BF16, tag="wgtTb")    # transposed gates [e, t] (seed lhsT)
            acc = big.tile([P, NT * O], F32, tag="acc")      # output accumulator

            # ---- gating logits, transposed layout [e, t] ----
            # gw chunk [128, 8] is the stationary operand (8-column LDWEIGHTS
            # is ~free) and the fp32 x chunks stream as the moving operand:
            # 16 big MMs instead of 64 tiny ones with 64 full LDWEIGHTS.
            # The two 512-wide halves live in the expert psum pool (only
            # partitions 0:E used) so the pool keeps 3 bufs for the stream.
            pslt = [psump.tile([P, 512], F32, tag=f"ps{th}", name=f"pslt{th}")
                    for th in range(2)]
            for k in range(NK):
                for th in range(2):
                    nc.tensor.matmul(
                        pslt[th][0:E, :],
                        lhsT=gw[:, k * E:(k + 1) * E],
                        rhs=xg[:, k * T + th * 512: k * T + (th + 1) * 512],
                        start=(k == 0),
                        stop=False,
                    )
            # gate bias: logitsT[e, t] += gb[e] * ones[t] via K=1 matmul
            for th in range(2):
                nc.tensor.matmul(
                    pslt[th][0:E, :],
                    lhsT=gbrow[0:1, :],
                    rhs=ones512[0:1, :],
                    start=False,
                    stop=True,
                )
            ltsb = small.tile([E, T], F32, tag="ltsb")
            for th in range(2):
                nc.vector.tensor_copy(ltsb[:, th * 512:(th + 1) * 512],
                                      pslt[th][0:E, :])

            # ---- softmax + top-2 per token tile ----
            for ti in range(NT):
                # transpose logitsT[:, tile] -> [t, e]
                ptt = psg.tile([P, E], F32, tag="ptt")
                nc.tensor.transpose(ptt[:, :], ltsb[:, ti * P:(ti + 1) * P],
                                    ident[0:E, 0:E])
                logits = tmpp.tile([P, E], F32, tag="logits")
                nc.vector.tensor_copy(logits[:, :], ptt[:, :])
                mx = tmpp.tile([P, 1], F32, tag="mx")
                nc.vector.tensor_reduce(mx[:, :], logits[:, :], axis=mybir.AxisListType.X, op=mybir.AluOpType.max)
                nmx = tmpp.tile([P, 1], F32, tag="nmx")
                nc.vector.tensor_scalar_mul(nmx[:, :], mx[:, :], -1.0)
                exps = tmpp.tile([P, E], F32, tag="exps")
                nc.scalar.activation(exps[:, :], logits[:, :], mybir.ActivationFunctionType.Exp,
                                     bias=nmx[:, 0:1], scale=1.0)
                ssum = tmpp.tile([P, 1], F32, tag="ssum")
                nc.vector.tensor_reduce(ssum[:, :], exps[:, :], axis=mybir.AxisListType.X, op=mybir.AluOpType.add)
                rinv = tmpp.tile([P, 1], F32, tag="rinv")
                nc.vector.reciprocal(rinv[:, :], ssum[:, :])
                probs = tmpp.tile([P, E], F32, tag="probs")
                nc.vector.tensor_scalar_mul(probs[:, :], exps[:, :], rinv[:, 0:1])
                srt = tmpp.tile([P, 8], F32, tag="srt")
                nc.vector.max(out=srt[:, :], in_=probs[:, :])
                # fused top-2 mask+apply: wgt = (probs >= 2nd-largest) * probs
                nc.vector.scalar_tensor_tensor(
                    out=wgt[:, ti * E:(ti + 1) * E], in0=probs[:, :],
                    scalar=srt[:, 1:2], in1=probs[:, :],
                    op0=mybir.AluOpType.is_ge, op1=mybir.AluOpType.mult,
                )
                # transpose the gate tile -> [E, P] for the expert-bias matmul
                pt = psg.tile([E, P], F32, tag="pt")
                nc.tensor.transpose(pt[:, :], wgt[:, ti * E:(ti + 1) * E], ident[:, :])
                nc.vector.tensor_copy(wgtTb[:, ti * P:(ti + 1) * P], pt[:, :])

            # ---- dense expert loop (bf16 operands, fp32 PSUM) ----
            # Expert 0 OVERWRITES acc (tensor_scalar, no acc read) so its
            # combine does not wait on anything but its own psum + gates.
            # The weighted expert bias (acc += sum_e w[t,e]*b_e) is emitted
            # after expert 1 so it rides mid-stream, off the critical path.
            def emit_bias_seed():
                for ti in range(NT):
                    for oi in range(NO):
                        psb = psump.tile([P, 512], F32, tag=f"ps{oi}", name=f"psb{oi}")
                        nc.tensor.matmul(
                            psb[:, :],
                            lhsT=wgtTb[:, ti * P:(ti + 1) * P],
                            rhs=ebt[:, oi * 512:(oi + 1) * 512],
                            start=True, stop=True,
                        )
                        col = acc[:, ti * O + oi * 512: ti * O + (oi + 1) * 512]
                        nc.vector.tensor_add(col, col, psb[:, :])

            for e in range(E):
                wte = wpool.tile([P, NK * O], BF16, tag="wte")
                # wTb is host-pretiled to [E, P, NK*O]: one trigger, 128
                # contiguous 16KB rows (DMA triggers cost ~700ns each on the
                # sync queue)
                nc.sync.dma_start(out=wte[:, :], in_=wTb[e])
                for ti in range(NT):
                    # two PSUM banks accumulate both o-halves off one stationary
                    # load per (ti, k)
                    pss = [psump.tile([P, 512], F32, tag=f"ps{oi}", name=f"pss{oi}") for oi in range(NO)]
                    for k in range(NK):
                        for oi in range(NO):
                            nc.tensor.matmul(
                                pss[oi][:, :],
                                lhsT=xbs[:, k * T + ti * P: k * T + (ti + 1) * P],
                                rhs=wte[:, k * O + oi * 512: k * O + oi * 512 + 512],
                                start=(k == 0),
                                stop=(k == NK - 1),
                            )
                    for oi in range(NO):
                        col = acc[:, ti * O + oi * 512: ti * O + (oi + 1) * 512]
                        wcol = wgt[:, ti * E + e: ti * E + e + 1]
                        if e == 0:
                            # overwrite: acc = psum * gate
                            nc.vector.tensor_scalar_mul(col, pss[oi][:, :], wcol)
                        else:
                            # fused one-pass combine: acc = psum * gate + acc
                            nc.vector.scalar_tensor_tensor(
                                out=col, in0=pss[oi][:, :], scalar=wcol, in1=col,
                                op0=mybir.AluOpType.mult, op1=mybir.AluOpType.add,
                            )
                        if e == E - 1:
                            # each o-half of acc[ti] is final as soon as its
                            # combine lands -- ship it without waiting for the
                            # other half (compresses the end-of-kernel drain)
                            nc.sync.dma_start(
                                out=y[ti * P:(ti + 1) * P, oi * 512:(oi + 1) * 512],
                                in_=col)
                if e == 1:
                    emit_bias_seed()

    _split_multi_waits(nc)
    return nc


def _split_multi_waits(nc):
    """This container's walrus rejects instructions carrying more than one
    on_wait semaphore condition ("Too many sync wait commands"). Move extra
    waits onto same-engine NoOp instructions inserted immediately before the
    instruction: the engine sequencer executes in program order, so blocking
    on the NoOps first is semantically identical."""
    nop_id = [0]
    for fn in nc.m.functions:
        for blk in fn.blocks:
            changed = False
            newinsts = []
            for inst in blk.instructions:
                si = getattr(inst, "sync_info", None)
                waits = list(si.on_wait) if si is not None and si.on_wait else []
                if len(waits) > 1:
                    changed = True
                    for w in waits[:-1]:
                        nop = mybir.InstNoOp(
                            name=f"I-waitnop-{nop_id[0]}", engine=inst.engine,
                            ins=[], outs=[],
                            sync_info=mybir.SyncInfo(on_wait=[w], on_update=[]),
                        )
                        nop_id[0] += 1
                        newinsts.append(nop)
                    inst.sync_info = mybir.SyncInfo(
                        on_wait=[waits[-1]], on_update=list(si.on_update))
                newinsts.append(inst)
            if changed:
                blk.instructions = newinsts


def kernel(x, gate_w, gate_b, expert_w, expert_b):
    x = np.ascontiguousarray(np.asarray(x, dtype=np.float32))
    gate_w = np.asarray(gate_w, dtype=np.float32)
    gate_b = np.asarray(gate_b, dtype=np.float32)
    expert_w = np.asarray(expert_w, dtype=np.float32)
    expert_b = np.asarray(expert_b, dtype=np.float32)

    B, S, _H = x.shape
    flat = x.reshape(B * S, _H)

    gwT = np.ascontiguousarray(gate_w.T)                      # [H, E]
    gbc = np.ascontiguousarray(gate_b.reshape(1, E))          # [1, E]
    # [E, H, O] -> SBUF tile layout [E, P, NK*O]: partition p holds chunk k
    # at columns [k*O, (k+1)*O), so one contiguous DMA per expert
    wTb = np.ascontiguousarray(
        expert_w.transpose(0, 2, 1).astype(ml_dtypes.bfloat16)
        .reshape(E, NK, P, O).transpose(0, 2, 1, 3).reshape(E, P, NK * O))
    ebb = np.ascontiguousarray(expert_b.astype(ml_dtypes.bfloat16))  # [E, O] bf16

    if "nc" not in _CACHE:
        _CACHE["nc"] = build_nc()
    nc = _CACHE["nc"]

    in_maps = []
    for c in range(N_CORES):
        shard = flat[c * T:(c + 1) * T]                       # [T, H]
        xTf = np.ascontiguousarray(shard.T)                   # [H, T] fp32
        in_maps.append({"xTf": xTf, "gwT": gwT, "gbc": gbc,
                        "wTb": wTb, "ebb": ebb})

    import os
    kw = {}
    if os.environ.get("KTRACE", "0") not in ("", "0"):
        kw = dict(trace=True, tmpdir=os.environ.get("KTRACE_DIR") or None)
    res = run_bass_kernel_spmd(nc, in_maps, core_ids=list(range(N_CORES)), **kw)
    out = np.concatenate([res.results[c]["y"] for c in range(N_CORES)], axis=0)
    _CACHE["last_exec_ns"] = res.exec_time_ns
    return out.reshape(B, S, O)
